# revision 3
# baseline (speedup 1.0000x reference)
"""MoE top-1 routing kernel for Trainium2 (8 NeuronCores).

Reference computation (B=8, S=1024, D=768, E=8, F=3072):
    gates = softmax(x @ gate_w + gate_b); expert_idx = argmax(gates)
    out[t] = gelu(x[t] @ w1[e] + b1[e]) @ w2[e] + b2[e]   for e = expert_idx[t]
    (no gate-probability scaling)

Strategy:
  * Routing on host in fp64 (softmax is monotonic, so argmax of logits ==
    argmax of gates; observed top-2 logit gaps are far above fp32 noise).
  * Feature-slice parallelism: every core holds the q-th 1/8 slice of the
    F dimension of ALL 8 experts' weights (w1 [D, F/8], w2 [F/8, D]) and
    processes ALL 8192 tokens, producing a partial sum of the second
    matmul.  The host adds the eight partials + b2.  This is perfectly
    load-balanced with ZERO padding (each core does exactly T*(D*F/8)*2*2
    MACs), unlike expert-parallel dispatch which pads to the max count.
  * Tokens are sorted by expert on the host (blocks ordered by ascending
    count so the first tiles are cheap to DMA); each block's tiles use
    that expert's weight slice.
  * Matmuls in bf16 with fp32 PSUM accumulation; activations stay
    transposed ([feature, token]).  gelu (erf-based) on the Scalar engine
    with the b1 bias fused; FFN2 partial-sums are copied PSUM->SBUF as
    bf16 on the Vector engine and DMA'd out.
  * DMA schedule: Scalar issues NO DMAs (so the activation-table loads
    and gelus are never queued behind transfers); the first block's
    x/w1/w2 pieces are split small and spread across the Sync/Vector/
    GpSimd/Tensor queues so they land while the PE warms up.
"""

import sys

try:
    import concourse  # noqa: F401
except ImportError:
    sys.path.insert(0, "/opt/trn_rl_repo")

import numpy as np
import ml_dtypes

import concourse.bass as bass  # noqa: F401
import concourse.tile as tile
import concourse.mybir as mybir
from concourse import bacc
from concourse import bass_utils

BF16 = mybir.dt.bfloat16
F32 = mybir.dt.float32
AF = mybir.ActivationFunctionType

B, S, D, E = 8, 1024, 768, 8
F = 4 * D           # 3072
T = B * S           # 8192
KD = D // 128       # 6 contraction chunks over D
FS = F // 8         # 384 features per core (1/8 slice)
KQ = FS // 128      # 3 chunks over the F-slice
N_CORES = 8
MAX_N = 512         # moving-dim tile (one fp32 PSUM bank)
N_WARM = 40         # PE-clock warmup matmuls

TRACE = False
LAST_RESULT = None


def _split_tiles(cap, lead=None, tail=None):
    """Split a block of `cap` tokens into near-equal tiles of <= MAX_N.
    `lead`/`tail` carve a small first/last tile (so the first matmuls
    depend on a sliver of DMA and the final output drain is short)."""
    if cap == 0:
        return []
    out = []
    off = 0
    tail_t = None
    if lead is not None and cap > lead + 128:
        out.append((0, lead))
        off = lead
        cap -= lead
    if tail is not None and cap > tail + 128:
        tail_t = tail
        cap -= tail
    n = -(-cap // MAX_N)
    base, rem = divmod(cap, n)
    for i in range(n):
        sz = base + (1 if i < rem else 0)
        out.append((off, sz))
        off += sz
    if tail_t is not None:
        out.append((off, tail_t))
    return out


def build_program(caps):
    """Per-core program: 8 expert blocks (ascending size) over all T tokens,
    each core computing its 1/8 F-slice partial output."""
    caps = list(caps)
    assert sum(caps) == T
    nc = bacc.Bacc("TRN2", target_bir_lowering=False, debug=False,
                   num_devices=N_CORES)

    xT_d = nc.dram_tensor("xT", (128, KD, T), BF16, kind="ExternalInput")
    w1_d = nc.dram_tensor("w1", (128, E, KQ, KD, 128), BF16,
                          kind="ExternalInput")
    w2_d = nc.dram_tensor("w2", (128, E, KD, KQ, 128), BF16,
                          kind="ExternalInput")
    b1_d = nc.dram_tensor("b1", (128, E, KQ), F32, kind="ExternalInput")
    yT_d = nc.dram_tensor("yT", (128, KD, T), BF16, kind="ExternalOutput")

    offs = np.concatenate([[0], np.cumsum(caps)]).astype(int)
    nz = [b for b in range(E) if caps[b] > 0]
    first_b, last_b = nz[0], nz[-1]

    # (block, tile-offset, width) in execution order.
    sched = []
    for b in nz:
        lead = 256 if b == first_b else None
        tail = 128 if b == last_b else None
        for (o, w) in _split_tiles(caps[b], lead=lead, tail=tail):
            sched.append((b, offs[b] + o, w))

    with tile.TileContext(nc) as tc:
        with (
            tc.tile_pool(name="wts", bufs=1) as wts,
            tc.tile_pool(name="act", bufs=2) as actp,
            tc.tile_pool(name="yp", bufs=3) as yp,
            tc.tile_pool(name="ps1", bufs=4, space="PSUM") as ps1,
            tc.tile_pool(name="ps2", bufs=4, space="PSUM") as ps2,
        ):
            xT = wts.tile([128, KD, T], BF16, tag="xT")
            w1 = wts.tile([128, E, KQ, KD, 128], BF16, tag="w1")
            w2 = wts.tile([128, E, KD, KQ, 128], BF16, tag="w2")
            b1 = wts.tile([128, E, KQ], F32, tag="b1")
            warm = wts.tile([128, 128], BF16, tag="warm")
            nc.gpsimd.memset(warm[:], 0.0)
            wps = ps1.tile([128, 128], F32, tag="ps1",
                           padded_shape=[128, MAX_N])

            # --- Head DMAs: first block's dependencies, split small and
            # spread over four queues so they land during PE warmup. ---
            e0 = first_b
            o0, w0 = sched[0][1], sched[0][2]
            # PE warmup: dummy matmuls flip the HAM clock gate to 2.4 GHz
            # while the head DMAs stream in.
            for _ in range(N_WARM):
                nc.tensor.matmul(wps[:, :], warm[:, :], warm[:, :])
            # GpSimd: b1 + the first block's w1 (per-m pieces so they run
            # on separate rings), then bulk weights per expert.
            nc.gpsimd.dma_start(b1[:], b1_d[:])
            for m in range(KQ):
                nc.gpsimd.dma_start(w1[:, e0, m, :, :], w1_d[:, e0, m, :, :])
            # Sync: first tile's tokens (per k pairs), then w2[e0], then
            # the remaining x of block e0, then bulk x per (block, k-pair).
            for k in range(0, KD, 2):
                nc.sync.dma_start(xT[:, k:k + 2, o0:o0 + w0],
                                  xT_d[:, k:k + 2, o0:o0 + w0])
            nc.sync.dma_start(w2[:, e0, :, :, :], w2_d[:, e0, :, :, :])
            if caps[e0] > w0:
                a, z = offs[e0] + w0, offs[e0 + 1]
                for k in range(0, KD, 2):
                    nc.sync.dma_start(xT[:, k:k + 2, a:z],
                                      xT_d[:, k:k + 2, a:z])
            # GpSimd: weights for the remaining blocks, interleaved per
            # expert in consumption order.
            for b in nz[1:]:
                for m in range(KQ):
                    nc.gpsimd.dma_start(w1[:, b, m, :, :],
                                        w1_d[:, b, m, :, :])
                nc.gpsimd.dma_start(w2[:, b, :, :, :], w2_d[:, b, :, :, :])
            # Sync: bulk x for remaining blocks, split per k-pair for ring
            # parallelism.
            for b in nz[1:]:
                a, z = offs[b], offs[b + 1]
                for k in range(0, KD, 2):
                    nc.sync.dma_start(xT[:, k:k + 2, a:z],
                                      xT_d[:, k:k + 2, a:z])

            def ffn1(b, n0, nt):
                h = actp.tile([128, KQ, nt], BF16, tag="h",
                              padded_shape=[128, KQ, MAX_N])
                for m in range(KQ):
                    ps = ps1.tile([128, nt], F32, tag="ps1",
                                  padded_shape=[128, MAX_N])
                    for k in range(KD):
                        nc.tensor.matmul(
                            ps[:, :],
                            w1[:, b, m, k, :],
                            xT[:, k, n0:n0 + nt],
                            start=(k == 0),
                            stop=(k == KD - 1),
                        )
                    nc.scalar.activation(h[:, m, :], ps[:, :], AF.Gelu,
                                         bias=b1[:, b, m:m + 1])
                return h

            out_q = [nc.sync, nc.gpsimd]

            def ffn2(b, n0, nt, h, ti, split_out=False):
                y = yp.tile([128, KD, nt], BF16, tag="y",
                            padded_shape=[128, KD, MAX_N])
                for md in range(KD):
                    ps = ps2.tile([128, nt], F32, tag="ps2",
                                  padded_shape=[128, MAX_N])
                    for k in range(KQ):
                        nc.tensor.matmul(
                            ps[:, :],
                            w2[:, b, md, k, :],
                            h[:, k, :],
                            start=(k == 0),
                            stop=(k == KQ - 1),
                        )
                    nc.vector.tensor_copy(y[:, md, :], ps[:, :])
                    if split_out:
                        out_q[md % 2].dma_start(yT_d[:, md, n0:n0 + nt],
                                                y[:, md, :])
                if not split_out:
                    out_q[ti % 2].dma_start(yT_d[:, :, n0:n0 + nt],
                                            y[:, :, :])

            # Software-pipelined emission: FFN1(t) ahead of FFN2(t-1) so the
            # PE never waits on the gelu of the tile it just produced.
            prev = None
            for ti, (b, n0, nt) in enumerate(sched):
                h = ffn1(b, n0, nt)
                if prev is not None:
                    ffn2(*prev)
                prev = (b, n0, nt, h, ti)
            if prev is not None:
                # Last tile: per-chunk output DMA so the transfers hide
                # under the final matmuls instead of trailing them.
                ffn2(*prev, split_out=True)

    nc.compile()
    return nc


_PROGRAM_CACHE = {}


def _get_program(caps):
    key = tuple(caps)
    if key not in _PROGRAM_CACHE:
        _PROGRAM_CACHE[key] = build_program(caps)
    return _PROGRAM_CACHE[key]


def kernel(x, gate_w, gate_b, w1, b1, w2, b2):
    x = np.asarray(x)
    w1 = np.asarray(w1)
    b1 = np.asarray(b1)
    w2 = np.asarray(w2)
    b2 = np.asarray(b2)
    xt = x.reshape(T, D)

    # --- Routing on host (fp64; softmax is monotonic => argmax of logits) ---
    logits = xt.astype(np.float64) @ np.asarray(gate_w, np.float64)
    logits += np.asarray(gate_b, np.float64)
    eidx = np.argmax(logits, axis=-1)
    counts = np.bincount(eidx, minlength=E)

    # Blocks in ascending-count order (small first tiles -> fast head DMA).
    perm = np.argsort(counts, kind="stable")
    caps = [int(counts[e]) for e in perm]
    offs = np.concatenate([[0], np.cumsum(caps)]).astype(int)

    nc = _get_program(caps)

    rank = np.empty(E, np.int64)
    rank[perm] = np.arange(E)
    order = np.argsort(rank[eidx], kind="stable")

    xt_bf = xt.astype(ml_dtypes.bfloat16)[order]           # [T, D] sorted
    # [T, D] -> [128, KD, T]
    xTg = np.ascontiguousarray(xt_bf.T.reshape(KD, 128, T).transpose(1, 0, 2))

    in_maps = [None] * N_CORES
    for q in range(N_CORES):
        w1q = np.empty((128, E, KQ, KD, 128), ml_dtypes.bfloat16)
        w2q = np.empty((128, E, KD, KQ, 128), ml_dtypes.bfloat16)
        b1q = np.empty((128, E, KQ), np.float32)
        for b, e in enumerate(perm):
            w1e = w1[e][:, q * FS:(q + 1) * FS]            # [D, FS]
            w1q[:, b] = w1e.reshape(KD, 128, KQ, 128).transpose(
                1, 2, 0, 3).astype(ml_dtypes.bfloat16)
            w2e = w2[e][q * FS:(q + 1) * FS, :]            # [FS, D]
            w2q[:, b] = w2e.reshape(KQ, 128, KD, 128).transpose(
                1, 2, 0, 3).astype(ml_dtypes.bfloat16)
            b1q[:, b] = b1[e][q * FS:(q + 1) * FS].reshape(KQ, 128).T
        in_maps[q] = {"xT": xTg, "w1": w1q, "w2": w2q, "b1": b1q}

    res = bass_utils.run_bass_kernel_spmd(nc, in_maps,
                                          core_ids=list(range(N_CORES)),
                                          trace=TRACE)
    global LAST_RESULT
    LAST_RESULT = res

    acc = res.results[0]["yT"].astype(np.float32)
    for q in range(1, N_CORES):
        acc += res.results[q]["yT"].astype(np.float32)
    # [128, KD, T] -> [T, D]
    yg = acc.transpose(1, 0, 2).reshape(D, T).T
    out = np.empty((T, D), np.float32)
    out[order] = yg + b2[eidx[order]]
    return out.reshape(B, S, D)


# revision 5
# speedup vs baseline: 1.0998x; 1.0998x over previous
"""MoE top-1 routing kernel for Trainium2 (8 NeuronCores).

Reference computation (B=8, S=1024, D=768, E=8, F=3072):
    gates = softmax(x @ gate_w + gate_b); expert_idx = argmax(gates)
    out[t] = gelu(x[t] @ w1[e] + b1[e]) @ w2[e] + b2[e]   for e = expert_idx[t]
    (no gate-probability scaling)

Strategy:
  * Routing on host in fp64 (softmax is monotonic, so argmax of logits ==
    argmax of gates; observed top-2 logit gaps are far above fp32 noise).
  * Every core holds a 1/4 slice of the F dimension of ALL 8 experts'
    weights and processes HALF the tokens: each expert's token block is
    split ceil/floor between core-rows {0-3} and {4-7}; core q in a row
    owns features [q*768, (q+1)*768).  Both rows see identical block
    capacities ceil(count/2) (odd blocks pad one dummy token), so one
    SPMD program serves all 8 cores with at most 8 tokens of padding
    total -- essentially perfect load balance.  The host adds the four
    F-slice partials per row + b2 and scatters rows back together.
  * Matmuls in bf16 with fp32 PSUM accumulation; activations stay
    transposed ([feature, token]).  gelu (erf-based) on the Scalar
    engine with the b1 bias fused; FFN2 partial-sums are copied
    PSUM->SBUF as bf16 on the Vector engine and DMA'd out.
  * DMA plan: the Scalar queue issues NO transfers (so the activation
    table load and gelus are never queued behind descriptor setup --
    DMA issue cost scales with descriptor rows).  Expert weights stream
    just-in-time through 3-deep ring buffers as single contiguous-row
    DMAs on GpSimd; token x arrives block-pair-granular on Sync; output
    tiles leave as cheap 2D per-feature-chunk DMAs on Sync/GpSimd.
"""

import sys

try:
    import concourse  # noqa: F401
except ImportError:
    sys.path.insert(0, "/opt/trn_rl_repo")

import numpy as np
import ml_dtypes

import concourse.bass as bass  # noqa: F401
import concourse.tile as tile
import concourse.mybir as mybir
from concourse import bacc
from concourse import bass_utils

BF16 = mybir.dt.bfloat16
F32 = mybir.dt.float32
AF = mybir.ActivationFunctionType

B, S, D, E = 8, 1024, 768, 8
F = 4 * D           # 3072
T = B * S           # 8192
KD = D // 128       # 6 contraction chunks over D
FQ = F // 4         # 768 features per core (1/4 slice)
KQ = FQ // 128      # 6 chunks over the F-slice
N_CORES = 8
MAX_N = 512         # moving-dim tile (one fp32 PSUM bank)
N_WARM = 40         # PE-clock warmup matmuls

TRACE = False
LAST_RESULT = None


def _split_tiles(cap, lead=None, tail=None):
    """Split a block of `cap` tokens into near-equal tiles of <= MAX_N.
    `lead`/`tail` carve a small first/last tile (so the first matmuls
    depend on a sliver of DMA and the final output drain is short)."""
    if cap == 0:
        return []
    out = []
    off = 0
    tail_t = None
    if lead is not None and cap > lead + 128:
        out.append((0, lead))
        off = lead
        cap -= lead
    if tail is not None and cap > tail + 128:
        tail_t = tail
        cap -= tail
    n = -(-cap // MAX_N)
    base, rem = divmod(cap, n)
    for i in range(n):
        sz = base + (1 if i < rem else 0)
        out.append((off, sz))
        off += sz
    if tail_t is not None:
        out.append((off, tail_t))
    return out


def build_program(caps):
    """Per-core program: 8 expert blocks with capacities `caps` (one half
    of each expert's tokens), F/4 feature slice of every expert."""
    caps = list(caps)
    CT = sum(caps)
    nc = bacc.Bacc("TRN2", target_bir_lowering=False, debug=False,
                   num_devices=N_CORES)

    xT_d = nc.dram_tensor("xT", (128, KD, CT), BF16, kind="ExternalInput")
    w1_d = nc.dram_tensor("w1", (128, E, KQ, KD, 128), BF16,
                          kind="ExternalInput")
    w2_d = nc.dram_tensor("w2", (128, E, KD, KQ, 128), BF16,
                          kind="ExternalInput")
    b1_d = nc.dram_tensor("b1", (128, E, KQ), F32, kind="ExternalInput")
    yT_d = nc.dram_tensor("yT", (128, KD, CT), BF16, kind="ExternalOutput")

    offs = np.concatenate([[0], np.cumsum(caps)]).astype(int)
    nz = [b for b in range(E) if caps[b] > 0]
    first_b, last_b = nz[0], nz[-1]

    # (block, tile-offset, width) in execution order.
    sched = []
    for b in nz:
        lead = 256 if b == first_b else None
        tail = 128 if b == last_b else None
        for (o, w) in _split_tiles(caps[b], lead=lead, tail=tail):
            sched.append((b, offs[b] + o, w))

    with tile.TileContext(nc) as tc:
        with (
            tc.tile_pool(name="wts", bufs=1) as wts,
            tc.tile_pool(name="w1p", bufs=3) as w1p,
            tc.tile_pool(name="w2p", bufs=3) as w2p,
            tc.tile_pool(name="act", bufs=2) as actp,
            tc.tile_pool(name="yp", bufs=3) as yp,
            tc.tile_pool(name="ps1", bufs=4, space="PSUM") as ps1,
            tc.tile_pool(name="ps2", bufs=4, space="PSUM") as ps2,
        ):
            xT = wts.tile([128, KD, CT], BF16, tag="xT")
            b1 = wts.tile([128, E, KQ], F32, tag="b1")
            warm = wts.tile([128, 128], BF16, tag="warm")
            nc.gpsimd.memset(warm[:], 0.0)
            wps = ps1.tile([128, 128], F32, tag="ps1",
                           padded_shape=[128, MAX_N])

            # PE warmup: dummy matmuls flip the HAM clock gate to 2.4 GHz
            # while the head DMAs stream in.
            for _ in range(N_WARM):
                nc.tensor.matmul(wps[:, :], warm[:, :], warm[:, :])

            w1t = {}
            w2t = {}

            def fetch(b, split=False):
                w1t[b] = w1p.tile([128, KQ, KD, 128], BF16, tag="w1e",
                                  name=f"w1e{b}")
                w2t[b] = w2p.tile([128, KD, KQ, 128], BF16, tag="w2e",
                                  name=f"w2e{b}")
                if split:
                    # First block: per-m pieces so the earliest-needed
                    # weights land first on parallel rings.
                    for m in range(KQ):
                        nc.gpsimd.dma_start(w1t[b][:, m], w1_d[:, b, m])
                    nc.sync.dma_start(w2t[b][:], w2_d[:, b])
                else:
                    nc.gpsimd.dma_start(w1t[b][:], w1_d[:, b])
                    nc.gpsimd.dma_start(w2t[b][:], w2_d[:, b])

            # --- Head DMAs ---
            e0 = first_b
            o0, w0 = sched[0][1], sched[0][2]
            nc.gpsimd.dma_start(b1[:], b1_d[:])
            fetch(e0, split=True)
            # Sync: first tile's tokens (per k-pair), then the rest of the
            # first block, then x per (block-pair, k-pair).
            for k in range(0, KD, 2):
                nc.sync.dma_start(xT[:, k:k + 2, o0:o0 + w0],
                                  xT_d[:, k:k + 2, o0:o0 + w0])
            if caps[e0] > w0:
                a, z = offs[e0] + w0, offs[e0 + 1]
                for k in range(0, KD, 2):
                    nc.sync.dma_start(xT[:, k:k + 2, a:z],
                                      xT_d[:, k:k + 2, a:z])
            if len(nz) > 1:
                fetch(nz[1])
            for i in range(1, len(nz), 2):
                a = offs[nz[i]]
                z = offs[nz[i + 1] + 1] if i + 1 < len(nz) else offs[nz[i] + 1]
                for k in range(0, KD, 2):
                    nc.sync.dma_start(xT[:, k:k + 2, a:z],
                                      xT_d[:, k:k + 2, a:z])

            def ffn1(b, n0, nt):
                h = actp.tile([128, KQ, nt], BF16, tag="h",
                              padded_shape=[128, KQ, MAX_N])
                for m in range(KQ):
                    ps = ps1.tile([128, nt], F32, tag="ps1",
                                  padded_shape=[128, MAX_N])
                    for k in range(KD):
                        nc.tensor.matmul(
                            ps[:, :],
                            w1t[b][:, m, k, :],
                            xT[:, k, n0:n0 + nt],
                            start=(k == 0),
                            stop=(k == KD - 1),
                        )
                    nc.scalar.activation(h[:, m, :], ps[:, :], AF.Gelu,
                                         bias=b1[:, b, m:m + 1])
                return h

            out_q = [nc.sync, nc.gpsimd]

            def ffn2(b, n0, nt, h):
                y = yp.tile([128, KD, nt], BF16, tag="y",
                            padded_shape=[128, KD, MAX_N])
                for md in range(KD):
                    ps = ps2.tile([128, nt], F32, tag="ps2",
                                  padded_shape=[128, MAX_N])
                    for k in range(KQ):
                        nc.tensor.matmul(
                            ps[:, :],
                            w2t[b][:, md, k, :],
                            h[:, k, :],
                            start=(k == 0),
                            stop=(k == KQ - 1),
                        )
                    nc.vector.tensor_copy(y[:, md, :], ps[:, :])
                    # Cheap 2D per-chunk output DMA, alternating queues.
                    out_q[md % 2].dma_start(yT_d[:, md, n0:n0 + nt],
                                            y[:, md, :])

            # Software-pipelined emission: FFN1(t) ahead of FFN2(t-1) so the
            # PE never waits on the gelu of the tile it just produced.
            # Weight blocks are prefetched two experts ahead through the
            # ring pools.
            prev = None
            cur_block = None
            for (b, n0, nt) in sched:
                if b != cur_block:
                    cur_block = b
                    bi = nz.index(b)
                    if bi + 2 < len(nz):
                        fetch(nz[bi + 2])
                h = ffn1(b, n0, nt)
                if prev is not None:
                    ffn2(*prev)
                prev = (b, n0, nt, h)
            if prev is not None:
                ffn2(*prev)

    nc.compile()
    return nc


_PROGRAM_CACHE = {}


def _get_program(caps):
    key = tuple(caps)
    if key not in _PROGRAM_CACHE:
        _PROGRAM_CACHE[key] = build_program(caps)
    return _PROGRAM_CACHE[key]


def kernel(x, gate_w, gate_b, w1, b1, w2, b2):
    x = np.asarray(x)
    w1 = np.asarray(w1)
    b1 = np.asarray(b1)
    w2 = np.asarray(w2)
    b2 = np.asarray(b2)
    xt = x.reshape(T, D)

    # --- Routing on host (fp64; softmax is monotonic => argmax of logits) ---
    logits = xt.astype(np.float64) @ np.asarray(gate_w, np.float64)
    logits += np.asarray(gate_b, np.float64)
    eidx = np.argmax(logits, axis=-1)
    counts = np.bincount(eidx, minlength=E)

    perm = np.argsort(counts, kind="stable")
    caps = [int(-(-counts[e] // 2)) for e in perm]   # ceil(count/2)
    CT = sum(caps)
    offs = np.concatenate([[0], np.cumsum(caps)]).astype(int)

    nc = _get_program(caps)

    xt_bf = xt.astype(ml_dtypes.bfloat16)
    idxA, idxB = [], []          # per block: token indices (B may be padded)
    realB = []                   # per block: number of REAL tokens in B half
    xA = np.zeros((CT, D), ml_dtypes.bfloat16)
    xB = np.zeros((CT, D), ml_dtypes.bfloat16)
    for bI, e in enumerate(perm):
        idx = np.nonzero(eidx == e)[0]
        nA = (len(idx) + 1) // 2
        a_idx, b_idx = idx[:nA], idx[nA:]
        o = offs[bI]
        xA[o:o + len(a_idx)] = xt_bf[a_idx]
        xB[o:o + len(b_idx)] = xt_bf[b_idx]
        idxA.append(a_idx)
        idxB.append(b_idx)
        realB.append(len(b_idx))

    def to_xT(xg):
        return np.ascontiguousarray(
            xg.T.reshape(KD, 128, CT).transpose(1, 0, 2))

    xTA, xTB = to_xT(xA), to_xT(xB)

    in_maps = [None] * N_CORES
    for q in range(4):
        w1q = np.empty((128, E, KQ, KD, 128), ml_dtypes.bfloat16)
        w2q = np.empty((128, E, KD, KQ, 128), ml_dtypes.bfloat16)
        b1q = np.empty((128, E, KQ), np.float32)
        for bI, e in enumerate(perm):
            w1e = w1[e][:, q * FQ:(q + 1) * FQ]            # [D, FQ]
            w1q[:, bI] = w1e.reshape(KD, 128, KQ, 128).transpose(
                1, 2, 0, 3).astype(ml_dtypes.bfloat16)
            w2e = w2[e][q * FQ:(q + 1) * FQ, :]            # [FQ, D]
            w2q[:, bI] = w2e.reshape(KQ, 128, KD, 128).transpose(
                1, 2, 0, 3).astype(ml_dtypes.bfloat16)
            b1q[:, bI] = b1[e][q * FQ:(q + 1) * FQ].reshape(KQ, 128).T
        in_maps[q] = {"xT": xTA, "w1": w1q, "w2": w2q, "b1": b1q}
        in_maps[4 + q] = {"xT": xTB, "w1": w1q, "w2": w2q, "b1": b1q}

    res = bass_utils.run_bass_kernel_spmd(nc, in_maps,
                                          core_ids=list(range(N_CORES)),
                                          trace=TRACE)
    global LAST_RESULT
    LAST_RESULT = res

    accA = res.results[0]["yT"].astype(np.float32)
    accB = res.results[4]["yT"].astype(np.float32)
    for q in range(1, 4):
        accA += res.results[q]["yT"].astype(np.float32)
        accB += res.results[4 + q]["yT"].astype(np.float32)
    # [128, KD, CT] -> [CT, D]
    ygA = accA.transpose(1, 0, 2).reshape(D, CT).T
    ygB = accB.transpose(1, 0, 2).reshape(D, CT).T

    out = np.empty((T, D), np.float32)
    for bI, e in enumerate(perm):
        o = offs[bI]
        ia, ib, nb = idxA[bI], idxB[bI], realB[bI]
        out[ia] = ygA[o:o + len(ia)] + b2[e]
        if nb:
            out[ib] = ygB[o:o + nb] + b2[e]
    return out.reshape(B, S, D)
